# revision 1
# baseline (speedup 1.0000x reference)
"""Masked dot-product attention (d2l DotProductAttention, inference) on 8
Trainium2 NeuronCores via Bass/Tile.

Math: out[b] = softmax(mask(Q[b] @ K[b].T / sqrt(64), valid_lens[b])) @ V[b]
for b in 0..31, seq 2048, head dim 64.

Strategy
--------
* Work is decomposed into PIECES: (head, subset of its 128-row k-tiles).
  Only k-tiles below a head's valid_len exist (sparsity: 280 of 512
  dense tiles for the seed-0 input). Pieces are packed into NSLOTS
  uniform segments per core x 8 cores so the single SPMD program's
  baked per-segment tile counts stay load balanced (C = sum of segment
  sizes ~= ceil(total_tiles/8)). A head split across pieces/cores is
  recombined on the host by summing the partial (unnormalized)
  numerators and denominators.
* On-chip, scores are computed transposed, S^T[k, q] (k on partitions):
  lhsT = K^T tile [64, 128], rhs = Q^T [64, 512]. Softmax needs no
  max-subtraction (|scores| <~ 6, exp cannot overflow fp32; the
  reference's -1e6 mask fill makes masked exp exactly 0), so
  P^T = exp(S^T / 8) is one ScalarE pass per k-tile, PSUM -> SBUF bf16.
* Masking + softmax denominator fold into the V operand on the host:
  vm = [V * m, m] (m = 0/1 mask). AV: lhsT = vm tile [128, 65],
  rhs = P^T [128, 512] accumulates O'[d, q] over the segment's k-tiles
  in PSUM; row 64 is the denominator. Masked/padded k rows contribute
  exactly 0 to numerator and denominator, so padded garbage scores are
  harmless. AV for tile t issues after exp of tile t+1 (one-tile
  software pipeline) so the PE never waits on the ScalarE.
* Host post-pass: per head, sum piece partials, out = (num/den).T.
  Host pre-transposes Q/K to [64, S] bf16 and pre-tiles vm so all DMAs
  are contiguous per partition.
"""

import math
import sys

import numpy as np

for _p in (
    "/root/.axon_site",
    "/root/.axon_site/_ro/trn_rl_repo",
    "/root/.axon_site/_ro/pypackages",
):
    if _p not in sys.path:
        sys.path.append(_p)

import ml_dtypes


def _ensure_axon_hooks_shim():
    """This image's antenv package lacks axon_hooks; bass_utils imports it
    when tracing is requested (e.g. via BASS_TRACE in the environment).
    Provide a null shim so tracing degrades gracefully instead of crashing.
    A harness can set a real hook via set_axon_ntff_profile_hook."""
    import types

    if "antenv.axon_hooks" in sys.modules:
        return
    try:
        import antenv.axon_hooks  # noqa: F401

        return
    except ImportError:
        pass
    import antenv  # noqa: F401

    mod = types.ModuleType("antenv.axon_hooks")
    mod._hook = None
    mod.set_axon_ntff_profile_hook = lambda h: setattr(mod, "_hook", h)
    mod.get_axon_ntff_profile_hook = lambda: mod._hook
    sys.modules["antenv.axon_hooks"] = mod


_ensure_axon_hooks_shim()

import concourse.bacc as bacc
import concourse.mybir as mybir
import concourse.tile as tile
from concourse.bass_utils import run_bass_kernel_spmd

N, S, D = 32, 2048, 64
N_CORES = 8
KTILE = 128           # k rows per tile (PE contraction tile)
VW = D + 1            # V columns + denominator ones-column

BF16 = mybir.dt.bfloat16
F32 = mybir.dt.float32

TRACE = False          # test.py flips this to profile
LAST_RESULTS = None    # BassKernelResults of the last run

_program_cache: dict = {}


def _build_program(T):
    """One SPMD program; T = per-segment k-tile counts (desc, same on all
    cores)."""
    nslots = len(T)
    maxpt = max(T)
    nc = bacc.Bacc("TRN2", target_bir_lowering=False, debug=False)
    qt = nc.dram_tensor("qt", [nslots, D, S], BF16, kind="ExternalInput")
    kt = nc.dram_tensor("kt", [nslots, D, maxpt * KTILE], BF16, kind="ExternalInput")
    vm = nc.dram_tensor("vm", [nslots, KTILE, maxpt * VW], BF16, kind="ExternalInput")
    out = nc.dram_tensor("out", [nslots, VW, S], F32, kind="ExternalOutput")

    exp = mybir.ActivationFunctionType.Exp
    scale = float(1.0 / math.sqrt(D))

    with tile.TileContext(nc) as tc:
        with (
            tc.tile_pool(name="qp", bufs=3) as qp,
            tc.tile_pool(name="kp", bufs=3) as kp,
            tc.tile_pool(name="vp", bufs=3) as vp,
            tc.tile_pool(name="pp", bufs=4) as pp,
            tc.tile_pool(name="ob", bufs=3) as ob,
            tc.tile_pool(name="sp", bufs=3, space="PSUM") as sp,
            tc.tile_pool(name="op", bufs=1, space="PSUM") as op,
        ):
            for j in range(nslots):
                tj = T[j]
                if tj == 0:
                    continue
                ktj = kp.tile([D, tj * KTILE], BF16, tag="kt")
                qtj = qp.tile([D, S], BF16, tag="qt")
                vmj = vp.tile([KTILE, tj * VW], BF16, tag="vm")
                if j == 0:
                    # expose the first matmul's operands as soon as possible:
                    # dependency tracking is per tile object, so the first
                    # k-tile of K^T and first q-chunk of Q^T get their own
                    # small DMAs ahead of the bulk loads
                    nc.sync.dma_start(ktj[:, :KTILE], kt[j, :, :KTILE])
                    nc.sync.dma_start(qtj[:, :512], qt[j, :, :512])
                    if tj > 1:
                        nc.sync.dma_start(
                            ktj[:, KTILE:], kt[j, :, KTILE : tj * KTILE]
                        )
                    nc.sync.dma_start(vmj[:, :], vm[j, :, : tj * VW])
                    for qq in range(1, S // 512):
                        nc.sync.dma_start(
                            qtj[:, qq * 512 : (qq + 1) * 512],
                            qt[j, :, qq * 512 : (qq + 1) * 512],
                        )
                else:
                    nc.sync.dma_start(ktj[:, :], kt[j, :, : tj * KTILE])
                    nc.sync.dma_start(qtj[:, :], qt[j])
                    nc.sync.dma_start(vmj[:, :], vm[j, :, : tj * VW])

                # two q-half passes over the segment's k-tiles: the score
                # PSUM tile triple-buffers (3 x 2 banks) + a 2-bank output
                # accumulator = 8 banks, so the next tiles' S matmuls never
                # wait on exp (ScalarE fully off the PE critical path)
                for half in range(2):
                    hq = half * (S // 2)
                    oacc = op.tile([VW, S // 2], F32, tag="o")
                    pending = None  # AV operands of the previous k-tile
                    for t in range(tj):
                        ps = sp.tile([KTILE, S // 2], F32, tag="s")
                        pt = pp.tile([KTILE, S // 2], BF16, tag="p")
                        for qq in range(2):
                            nc.tensor.matmul(
                                ps[:, qq * 512 : (qq + 1) * 512],
                                ktj[:, t * KTILE : (t + 1) * KTILE],
                                qtj[:, hq + qq * 512 : hq + (qq + 1) * 512],
                                start=True,
                                stop=True,
                            )
                        nc.scalar.activation(pt[:, :], ps[:, :], exp, scale=scale)
                        if pending is not None:
                            _emit_av(nc, oacc, *pending)
                        pending = (vmj, pt, t, tj)

                    # final k-tile: each 512-chunk's accumulation closes at
                    # its own AV matmul, so drain chunk-by-chunk behind it
                    vmj_l, pt_l, t_l, tj_l = pending
                    obuf = ob.tile([VW, S // 2], F32, tag="ob")
                    for qq in range(2):
                        qs = slice(qq * 512, (qq + 1) * 512)
                        nc.tensor.matmul(
                            oacc[:, qs],
                            vmj_l[:, t_l * VW : (t_l + 1) * VW],
                            pt_l[:, qs],
                            start=(t_l == 0),
                            stop=True,
                        )
                        nc.vector.tensor_copy(obuf[:, qs], oacc[:, qs])
                        # output DMAs on the GpSimd queue so the Sync queue
                        # only carries input prefetches
                        nc.gpsimd.dma_start(
                            out[j, :, hq + qq * 512 : hq + (qq + 1) * 512],
                            obuf[:, qs],
                        )
    nc.compile()
    return nc


def _emit_av(nc, oacc, vmj, pt, t, tj):
    for qq in range(2):
        nc.tensor.matmul(
            oacc[:, qq * 512 : (qq + 1) * 512],
            vmj[:, t * VW : (t + 1) * VW],
            pt[:, qq * 512 : (qq + 1) * 512],
            start=(t == 0),
            stop=(t == tj - 1),
        )


def _pack_pieces(tiles_per_head):
    """Split heads into 8*nslots pieces, minimizing C = sum of per-slot
    maxima (the baked per-core tile count). Returns (slot_sizes, pieces)
    where pieces[j][c] = (head, [tile indices]) for slot j, core c."""
    nheads = len(tiles_per_head)
    best = None
    for nslots in range(max(1, nheads // 8), nheads // 8 + 8):
        npieces = 8 * nslots
        n = dict.fromkeys(range(nheads), 1)

        def maxpiece(h):
            return math.ceil(tiles_per_head[h] / n[h])

        for _ in range(npieces - nheads):
            h = max(range(nheads), key=lambda h: (maxpiece(h), tiles_per_head[h]))
            if maxpiece(h) <= 1:
                break
            n[h] += 1
        pieces = []
        for h in range(nheads):
            nh = n[h]
            q, r = divmod(tiles_per_head[h], nh)
            start = 0
            for i in range(nh):
                sz = q + 1 if i < r else q
                if sz > 0:
                    pieces.append((sz, h, list(range(start, start + sz))))
                start += sz
        pieces.sort(key=lambda p: -p[0])
        while len(pieces) < npieces:
            pieces.append((0, -1, []))
        slot_sizes = tuple(pieces[8 * j][0] for j in range(nslots))
        C = sum(slot_sizes)
        # small penalty per extra slot for segment-drain overhead
        cost = C + 0.2 * nslots
        if best is None or cost < best[0]:
            best = (cost, slot_sizes, pieces)
    _, slot_sizes, pieces = best
    nslots = len(slot_sizes)
    grid = [[pieces[8 * j + c] for c in range(8)] for j in range(nslots)]
    return slot_sizes, grid


def kernel(queries, keys, values, valid_lens):
    global LAST_RESULTS
    queries = np.asarray(queries, dtype=np.float32)
    keys = np.asarray(keys, dtype=np.float32)
    values = np.asarray(values, dtype=np.float32)
    vl = np.asarray(valid_lens).astype(np.int64)
    assert queries.shape == (N, S, D) and vl.shape == (N,)

    tiles_per_head = [max(1, int(math.ceil(int(v) / KTILE))) for v in vl]
    slot_sizes, grid = _pack_pieces(tiles_per_head)
    nslots = len(slot_sizes)
    maxpt = max(slot_sizes)

    nc = _program_cache.get(slot_sizes)
    if nc is None:
        nc = _build_program(slot_sizes)
        _program_cache[slot_sizes] = nc

    bf = ml_dtypes.bfloat16
    qt_all = np.ascontiguousarray(queries.transpose(0, 2, 1)).astype(bf)  # [N,64,S]
    kt_all = np.ascontiguousarray(keys.transpose(0, 2, 1)).astype(bf)    # [N,64,S]
    # vm_all[h]: [KTILE, 16, VW]  (partition-major tiling of [V*m, m])
    vm_all = np.zeros((N, KTILE, S // KTILE, VW), dtype=bf)
    for h in range(N):
        m = (np.arange(S) < vl[h]).astype(np.float32)
        vp_full = np.concatenate([values[h] * m[:, None], m[:, None]], axis=1)
        vm_all[h] = vp_full.reshape(S // KTILE, KTILE, VW).transpose(1, 0, 2).astype(bf)

    in_maps = []
    for c in range(N_CORES):
        qt_c = np.zeros((nslots, D, S), dtype=bf)
        kt_c = np.zeros((nslots, D, maxpt * KTILE), dtype=bf)
        vm_c = np.zeros((nslots, KTILE, maxpt * VW), dtype=bf)
        for j in range(nslots):
            sz, h, tidx = grid[j][c]
            if sz == 0:
                continue
            qt_c[j] = qt_all[h]
            for i, t in enumerate(tidx):
                kt_c[j, :, i * KTILE : (i + 1) * KTILE] = kt_all[
                    h, :, t * KTILE : (t + 1) * KTILE
                ]
                vm_c[j, :, i * VW : (i + 1) * VW] = vm_all[h, :, t, :]
        in_maps.append({"qt": qt_c, "kt": kt_c, "vm": vm_c})

    res = run_bass_kernel_spmd(nc, in_maps, core_ids=list(range(N_CORES)), trace=TRACE)
    LAST_RESULTS = res

    acc = np.zeros((N, VW, S), dtype=np.float64)
    for c in range(N_CORES):
        o = res.results[c]["out"]  # [nslots, 65, S] fp32
        for j in range(nslots):
            sz, h, _ = grid[j][c]
            if sz > 0:
                acc[h] += o[j]
    out_full = (acc[:, :D, :] / acc[:, D : D + 1, :]).transpose(0, 2, 1)
    return np.ascontiguousarray(out_full.astype(np.float32))

